# revision 9
# baseline (speedup 1.0000x reference)
"""DiceLoss kernel for Trainium2 (8 NeuronCores, batch-sharded).

Math per image (H=W=1024):
    s  = sigmoid(output)
    P  = avgpool31x31(target)            (zero-padded box sum / 961)
    w  = 1 + 5*|P - target|
    inter = sum(s*t*w);  mask = sum((t+s)*w)
    loss  = 1 - (2*inter + 1e-3) / (mask + 1e-3);  final = mean over batch

Device strategy (per core: 4 images, 8 row-tiles of [128, 1024] each,
processed in pairs of row-tiles = [128, 2048] working set). All inputs
arrive as bf16 (host casts), halving HBM traffic:
  - horizontal 31-tap box: ONE tensor_tensor_scan per row-tile
        state = (t[x+15] + state) - t[x-16]   (fp32 state) over padded rows
  - vertical 31-tap box and the "- 961*t" subtraction: PE band matmuls
        psum = Bdiag@H_i + Babove@H_{i-1} + Bbelow@H_{i+1}
               - 960*I@t_i - 1*I@t_i        (both exact in bf16)
  - w1 = |5/961 * psum| on ACT (scale fused into Abs), bf16 out
  - fused DVE products with built-in row reductions (accum_out):
        u  = (w1+1)*t   with accum -> sum(t*w)    [scalar_tensor_tensor]
        v  = (w1+1)*s   with accum -> sum(s*w)    [scalar_tensor_tensor]
        p2 = s*u        with accum -> sum(s*t*w)  [tensor_tensor_reduce]
  - per-pair per-partition sums land in a [128, 48] f32 stash; host does
    the final cross-partition/cross-pair sums and the loss arithmetic.
No GPSIMD ops (they run ~25x below spec and lock the SBUF port shared
with DVE), no on-device casts, no PE reduction matmuls.
"""

import numpy as np

B, IMH, IMW = 32, 1024, 1024
NCORES = 8
IMGS = B // NCORES  # 4 images per core
NT = IMH // 128     # 8 row-tiles per image
NPAIR = NT // 2     # 4 pairs per image
PADW = 1072         # 32 zeros | 1024 data | 16 zeros (host-padded; 4B-aligned data)
SMOOTH = 1e-3

_CACHE = {}


def _make_consts():
    k = np.arange(128)[:, None]  # lhsT row (contraction index within rhs block)
    m = np.arange(128)[None, :]  # lhsT col (output partition within out block)
    diag = (np.abs(m - k) <= 15).astype(np.float32)
    above = (np.abs(128 + m - k) <= 15).astype(np.float32)   # rhs = H_{i-1}
    below = (np.abs(-128 + m - k) <= 15).astype(np.float32)  # rhs = H_{i+1}
    wband = np.stack([diag, above, below])                   # [3,128,128]
    wident = np.stack(
        [-960.0 * np.eye(128), -1.0 * np.eye(128)]
    ).astype(np.float32)                                     # [2,128,128]
    return wband, wident


def _build_program():
    from contextlib import ExitStack

    import concourse.bacc as bacc
    import concourse.tile as tile
    from concourse import mybir

    f32 = mybir.dt.float32
    bf16 = mybir.dt.bfloat16
    AL = mybir.AluOpType
    AF = mybir.ActivationFunctionType

    nc = bacc.Bacc("TRN2", target_bir_lowering=False, debug=False)
    t_dram = nc.dram_tensor("target", [IMGS, IMH, PADW], bf16, kind="ExternalInput")
    o_dram = nc.dram_tensor("outp", [IMGS, IMH, IMW], bf16, kind="ExternalInput")
    wb_dram = nc.dram_tensor("wband", [3, 128, 128], bf16, kind="ExternalInput")
    wi_dram = nc.dram_tensor("wident", [2, 128, 128], bf16, kind="ExternalInput")
    st_dram = nc.dram_tensor("stats", [128, IMGS * NPAIR * 3], f32, kind="ExternalOutput")

    with tile.TileContext(nc) as tc, ExitStack() as ctx:
        consts = ctx.enter_context(tc.tile_pool(name="consts", bufs=1))
        tpool = ctx.enter_context(tc.tile_pool(name="tbuf", bufs=3))
        opool = ctx.enter_context(tc.tile_pool(name="obuf", bufs=3))
        hpool = ctx.enter_context(tc.tile_pool(name="hbuf", bufs=5))
        spool = ctx.enter_context(tc.tile_pool(name="sbuf16", bufs=3))
        w1pool = ctx.enter_context(tc.tile_pool(name="w1buf", bufs=2))
        upool = ctx.enter_context(tc.tile_pool(name="ubuf", bufs=2))
        junkpool = ctx.enter_context(tc.tile_pool(name="junk", bufs=2))
        psd = ctx.enter_context(tc.tile_pool(name="psd", bufs=2, space="PSUM"))
        stpool = ctx.enter_context(tc.tile_pool(name="stout", bufs=1))

        wband = consts.tile([128, 3, 128], bf16, tag="wband")
        nc.sync.dma_start(wband[:], wb_dram.rearrange("a b c -> b a c"))
        wident = consts.tile([128, 2, 128], bf16, tag="wident")
        nc.sync.dma_start(wident[:], wi_dram.rearrange("a b c -> b a c"))

        stash = stpool.tile([128, IMGS * NPAIR, 3], f32, tag="stash")

        # ---- pipeline over (image, pair) with 1-pair lag ----
        seq = [(g, p) for g in range(IMGS) for p in range(NPAIR)]
        tbs, hbs, sbs = {}, {}, {}

        def emit_load(idx, g, p):
            tb = tpool.tile([128, 2, PADW], bf16, tag="tb")
            tsrc = t_dram[g, 256 * p : 256 * (p + 1), :].rearrange(
                "(two r) w -> r two w", two=2
            )
            nc.sync.dma_start(tb[:], tsrc)
            ob = opool.tile([128, 2, 1024], bf16, tag="ob")
            osrc = o_dram[g, 256 * p : 256 * (p + 1), :].rearrange(
                "(two r) w -> r two w", two=2
            )
            nc.sync.dma_start(ob[:], osrc)

            hb = hpool.tile([128, 2, 1040], bf16, tag="hb")
            for k in range(2):
                # out[c] = CUM(c) - CUM(c-31); cols 15+x hold H[x]
                nc.vector.tensor_tensor_scan(
                    out=hb[:, k, 0:1039],
                    data0=tb[:, k, 32:1071],
                    data1=tb[:, k, 1:1040],
                    initial=0.0,
                    op0=AL.add,
                    op1=AL.subtract,
                )
            sb = spool.tile([128, 2, 1024], bf16, tag="sb")
            nc.scalar.activation(sb[:], ob[:], AF.Sigmoid)
            tbs[idx], hbs[idx], sbs[idx] = tb, hb, sb

        def h_view(base, j, h):
            # H row-tile j (pair handle at seq index base + j//2), half h
            return hbs[base + j // 2][:, j % 2, 15 + 512 * h : 15 + 512 * (h + 1)]

        def emit_process(idx, g, p):
            base = idx - p  # seq index of this image's pair 0
            tb = tbs[idx]
            sb = sbs[idx]
            w1 = w1pool.tile([128, 2, 1024], bf16, tag="w1")
            dps = psd.tile([128, 4, 512], f32, tag="dps")
            for k in range(2):
                j = 2 * p + k
                for h in range(2):
                    q = 2 * k + h
                    nc.tensor.matmul(
                        dps[:, q, :], wband[:, 0, :], h_view(base, j, h),
                        start=True, stop=False,
                    )
                    if j > 0:
                        nc.tensor.matmul(
                            dps[:, q, :], wband[:, 1, :], h_view(base, j - 1, h),
                            start=False, stop=False,
                        )
                    if j < NT - 1:
                        nc.tensor.matmul(
                            dps[:, q, :], wband[:, 2, :], h_view(base, j + 1, h),
                            start=False, stop=False,
                        )
                    tview = tb[:, k, 32 + 512 * h : 32 + 512 * (h + 1)]
                    nc.tensor.matmul(
                        dps[:, q, :], wident[:, 0, :], tview, start=False, stop=False
                    )
                    nc.tensor.matmul(
                        dps[:, q, :], wident[:, 1, :], tview, start=False, stop=True
                    )
            nc.scalar.activation(
                out=w1[:].rearrange("p a b -> p (a b)"),
                in_=dps[:].rearrange("p a b -> p (a b)"),
                func=AF.Abs,
                scale=5.0 / 961.0,
            )

            pair = g * NPAIR + p
            tf = tb[:, :, 32:1056]
            u = upool.tile([128, 2, 1024], bf16, tag="u")
            nc.vector.scalar_tensor_tensor(
                out=u[:],
                in0=w1[:], scalar=1.0, in1=tf,
                op0=AL.add, op1=AL.mult,
                accum_out=stash[:, pair, 0:1],
            )
            v = junkpool.tile([128, 2, 1024], bf16, tag="v")
            nc.vector.scalar_tensor_tensor(
                out=v[:],
                in0=w1[:], scalar=1.0, in1=sb[:],
                op0=AL.add, op1=AL.mult,
                accum_out=stash[:, pair, 1:2],
            )
            p2 = junkpool.tile([128, 2, 1024], bf16, tag="p2")
            nc.vector.scalar_tensor_tensor(
                out=p2[:],
                in0=sb[:], scalar=1.0, in1=u[:],
                op0=AL.mult, op1=AL.mult,
                accum_out=stash[:, pair, 2:3],
            )

        for idx in range(len(seq) + 1):
            if idx < len(seq):
                emit_load(idx, *seq[idx])
            if idx >= 1:
                emit_process(idx - 1, *seq[idx - 1])

        nc.sync.dma_start(
            st_dram[:], stash[:].rearrange("p a b -> p (a b)")
        )

    nc.compile()
    return nc


def _get_program():
    key = "nc"
    if key not in _CACHE:
        _CACHE[key] = _build_program()
    return _CACHE[key]


def run_on_device(in_maps, **kwargs):
    from concourse.bass_utils import run_bass_kernel_spmd

    nc = _get_program()
    return run_bass_kernel_spmd(nc, in_maps, core_ids=list(range(NCORES)), **kwargs)


def make_in_maps(output, target):
    import ml_dtypes

    bf16 = ml_dtypes.bfloat16
    output = np.asarray(output, dtype=np.float32)
    target = np.asarray(target, dtype=np.float32)
    wband, wident = _make_consts()
    in_maps = []
    tpad = np.zeros((B, IMH, PADW), dtype=bf16)
    tpad[:, :, 32:1056] = target[:, 0].astype(bf16)
    o16 = output[:, 0].astype(bf16)
    wband16 = wband.astype(bf16)
    wident16 = wident.astype(bf16)
    for c in range(NCORES):
        in_maps.append(
            {
                "target": tpad[c * IMGS : (c + 1) * IMGS],
                "outp": np.ascontiguousarray(o16[c * IMGS : (c + 1) * IMGS]),
                "wband": wband16,
                "wident": wident16,
            }
        )
    return in_maps


def finish_on_host(results):
    losses = []
    for c in range(NCORES):
        st = np.asarray(results[c]["stats"], dtype=np.float64)  # [128, 48]
        st = st.reshape(128, IMGS, NPAIR, 3).sum(axis=(0, 2))   # [IMGS, 3]
        for g in range(IMGS):
            su, sv, sp2 = st[g]
            inter = sp2
            mask = su + sv
            losses.append(1.0 - (2.0 * inter + SMOOTH) / (mask + SMOOTH))
    return np.float32(np.mean(losses))


def kernel(output, target):
    in_maps = make_in_maps(output, target)
    res = run_on_device(in_maps)
    return finish_on_host(res.results)


# revision 14
# speedup vs baseline: 1.1475x; 1.1475x over previous
"""DiceLoss kernel for Trainium2 (8 NeuronCores, batch-sharded).

Math per image (H=W=1024):
    s  = sigmoid(output)
    P  = avgpool31x31(target)            (zero-padded box sum / 961)
    w  = 1 + 5*|P - target|
    inter = sum(s*t*w);  mask = sum((t+s)*w)
    loss  = 1 - (2*inter + 1e-3) / (mask + 1e-3);  final = mean over batch

Device strategy (per core: 4 images, 8 row-tiles of [128, 1024] each,
processed in pairs of row-tiles = [128, 2048] working set). All inputs
arrive as bf16 (host casts), halving HBM traffic:
  - horizontal 31-tap box: ONE tensor_tensor_scan per row-tile
        state = (t[x+15] + state) - t[x-16]   (fp32 state) over padded rows
  - vertical 31-tap box and the "- 961*t" subtraction: PE band matmuls
        psum = Bdiag@H_i + Babove@H_{i-1} + Bbelow@H_{i+1}
               - 960*I@t_i - 1*I@t_i        (both exact in bf16)
  - w1 = |5/961 * psum| on ACT (scale fused into Abs), bf16 out
  - fused DVE products with built-in row reductions (accum_out):
        u  = (w1+1)*t   with accum -> sum(t*w)    [scalar_tensor_tensor]
        v  = (w1+1)*s   with accum -> sum(s*w)    [scalar_tensor_tensor]
        p2 = s*u        with accum -> sum(s*t*w)  [tensor_tensor_reduce]
  - per-pair per-partition sums land in a [128, 48] f32 stash; host does
    the final cross-partition/cross-pair sums and the loss arithmetic.
No GPSIMD ops (they run ~25x below spec and lock the SBUF port shared
with DVE), no on-device casts, no PE reduction matmuls.
"""

import numpy as np

B, IMH, IMW = 32, 1024, 1024
NCORES = 8
IMGS = B // NCORES  # 4 images per core
NT = IMH // 128     # 8 row-tiles per image
NPAIR = NT // 2     # 4 pairs per image
PADW = 1072         # 32 zeros | 1024 data | 16 zeros (host-padded; 4B-aligned data)
SMOOTH = 1e-3

_CACHE = {}


def _make_consts():
    k = np.arange(128)[:, None]  # lhsT row (contraction index within rhs block)
    m = np.arange(128)[None, :]  # lhsT col (output partition within out block)
    diag = (np.abs(m - k) <= 15).astype(np.float32)
    above = (np.abs(128 + m - k) <= 15).astype(np.float32)   # rhs = H_{i-1}
    below = (np.abs(-128 + m - k) <= 15).astype(np.float32)  # rhs = H_{i+1}
    wband = np.stack([diag, above, below])                   # [3,128,128]
    wident = np.stack(
        [-960.0 * np.eye(128), -1.0 * np.eye(128)]
    ).astype(np.float32)                                     # [2,128,128]
    return wband, wident


def _build_program():
    from contextlib import ExitStack

    import concourse.bacc as bacc
    import concourse.tile as tile
    from concourse import mybir

    f32 = mybir.dt.float32
    bf16 = mybir.dt.bfloat16
    AL = mybir.AluOpType
    AF = mybir.ActivationFunctionType

    nc = bacc.Bacc("TRN2", target_bir_lowering=False, debug=False)
    t_dram = nc.dram_tensor("target", [IMGS, IMH, PADW], bf16, kind="ExternalInput")
    o_dram = nc.dram_tensor("outp", [IMGS, IMH, IMW], bf16, kind="ExternalInput")
    wb_dram = nc.dram_tensor("wband", [3, 128, 128], bf16, kind="ExternalInput")
    wi_dram = nc.dram_tensor("wident", [2, 128, 128], bf16, kind="ExternalInput")
    st_dram = nc.dram_tensor("stats", [IMGS, 3, 512], f32, kind="ExternalOutput")

    with tile.TileContext(nc) as tc, ExitStack() as ctx:
        consts = ctx.enter_context(tc.tile_pool(name="consts", bufs=1))
        tpool = ctx.enter_context(tc.tile_pool(name="tbuf", bufs=3))
        opool = ctx.enter_context(tc.tile_pool(name="obuf", bufs=3))
        hpool = ctx.enter_context(tc.tile_pool(name="hbuf", bufs=5))
        spool = ctx.enter_context(tc.tile_pool(name="sbuf16", bufs=3))
        w1pool = ctx.enter_context(tc.tile_pool(name="w1buf", bufs=2))
        wtpool = ctx.enter_context(tc.tile_pool(name="wtbuf", bufs=2))
        upool = ctx.enter_context(tc.tile_pool(name="ubuf", bufs=2))
        junkpool = ctx.enter_context(tc.tile_pool(name="junk", bufs=2))
        psd = ctx.enter_context(tc.tile_pool(name="psd", bufs=2, space="PSUM"))
        psr = ctx.enter_context(tc.tile_pool(name="psr", bufs=1, space="PSUM"))
        stpool = ctx.enter_context(tc.tile_pool(name="stout", bufs=2))

        wband = consts.tile([128, 3, 128], bf16, tag="wband")
        nc.sync.dma_start(wband[:], wb_dram.rearrange("a b c -> b a c"))
        wident = consts.tile([128, 2, 128], bf16, tag="wident")
        nc.sync.dma_start(wident[:], wi_dram.rearrange("a b c -> b a c"))
        ones16 = consts.tile([128, 1], bf16, tag="ones16")
        nc.vector.memset(ones16[:], 1.0)

        # ---- pipeline over (image, pair) with 1-pair lag ----
        seq = [(g, p) for g in range(IMGS) for p in range(NPAIR)]
        tbs, hbs, sbs = {}, {}, {}
        red = {}  # per-image [1,3,512] psum accumulators

        def emit_load(idx, g, p):
            tb = tpool.tile([128, 2, PADW], bf16, tag="tb")
            tsrc = t_dram[g, 256 * p : 256 * (p + 1), :].rearrange(
                "(two r) w -> r two w", two=2
            )
            nc.sync.dma_start(tb[:], tsrc)
            ob = opool.tile([128, 2, 1024], bf16, tag="ob")
            osrc = o_dram[g, 256 * p : 256 * (p + 1), :].rearrange(
                "(two r) w -> r two w", two=2
            )
            nc.sync.dma_start(ob[:], osrc)

            hb = hpool.tile([128, 2, 1040], bf16, tag="hb")
            for k in range(2):
                # out[c] = CUM(c) - CUM(c-31); cols 15+x hold H[x]
                nc.vector.tensor_tensor_scan(
                    out=hb[:, k, 0:1039],
                    data0=tb[:, k, 32:1071],
                    data1=tb[:, k, 1:1040],
                    initial=0.0,
                    op0=AL.add,
                    op1=AL.subtract,
                )
            sb = spool.tile([128, 2, 1024], bf16, tag="sb")
            nc.scalar.activation(sb[:], ob[:], AF.Sigmoid)
            tbs[idx], hbs[idx], sbs[idx] = tb, hb, sb

        def h_view(base, j, h):
            # H row-tile j (pair handle at seq index base + j//2), half h
            return hbs[base + j // 2][:, j % 2, 15 + 512 * h : 15 + 512 * (h + 1)]

        def emit_process(idx, g, p):
            base = idx - p  # seq index of this image's pair 0
            tb = tbs[idx]
            sb = sbs[idx]
            w1 = w1pool.tile([128, 2, 1024], bf16, tag="w1")
            for k in range(2):
                j = 2 * p + k
                dps = psd.tile([128, 2, 512], f32, tag="dps")
                for h in range(2):
                    nc.tensor.matmul(
                        dps[:, h, :], wband[:, 0, :], h_view(base, j, h),
                        start=True, stop=False,
                    )
                    if j > 0:
                        nc.tensor.matmul(
                            dps[:, h, :], wband[:, 1, :], h_view(base, j - 1, h),
                            start=False, stop=False,
                        )
                    if j < NT - 1:
                        nc.tensor.matmul(
                            dps[:, h, :], wband[:, 2, :], h_view(base, j + 1, h),
                            start=False, stop=False,
                        )
                    tview = tb[:, k, 32 + 512 * h : 32 + 512 * (h + 1)]
                    nc.tensor.matmul(
                        dps[:, h, :], wident[:, 0, :], tview, start=False, stop=False
                    )
                    nc.tensor.matmul(
                        dps[:, h, :], wident[:, 1, :], tview, start=False, stop=True
                    )
                nc.scalar.activation(
                    out=w1[:, k, :],
                    in_=dps[:].rearrange("p a b -> p (a b)"),
                    func=AF.Abs,
                    scale=5.0 / 961.0,
                )

            wt = wtpool.tile([128, 2, 1024], bf16, tag="wt")
            nc.vector.tensor_scalar_add(wt[:], w1[:], 1.0)
            tf = tb[:, :, 32:1056]
            u = upool.tile([128, 2, 1024], bf16, tag="u")
            nc.vector.tensor_mul(u[:], tf, wt[:])
            v = junkpool.tile([128, 2, 1024], bf16, tag="v")
            nc.vector.tensor_mul(v[:], sb[:], wt[:])
            p2 = junkpool.tile([128, 2, 1024], bf16, tag="p2")
            nc.vector.tensor_mul(p2[:], sb[:], u[:])

            if p == 0:
                red[g] = psr.tile([1, 3, 512], f32, name=f"red{g}", tag="red")
            for q, src in enumerate((u, v, p2)):
                sf = src[:].rearrange("p a b -> p (a b)")
                for c in range(4):
                    nc.tensor.matmul(
                        red[g][:, q, :],
                        ones16[:],
                        sf[:, 512 * c : 512 * (c + 1)],
                        start=(p == 0 and c == 0),
                        stop=(p == NPAIR - 1 and c == 3),
                        skip_group_check=True,
                    )
            if p == NPAIR - 1:
                st = stpool.tile([1, 3, 512], f32, tag="st")
                nc.scalar.copy(st[:], red[g][:])
                nc.sync.dma_start(st_dram[g : g + 1], st[:])

        for idx in range(len(seq) + 1):
            if idx < len(seq):
                emit_load(idx, *seq[idx])
            if idx >= 1:
                emit_process(idx - 1, *seq[idx - 1])

    nc.compile()
    return nc


def _get_program():
    key = "nc"
    if key not in _CACHE:
        _CACHE[key] = _build_program()
    return _CACHE[key]


def run_on_device(in_maps, **kwargs):
    from concourse.bass_utils import run_bass_kernel_spmd

    nc = _get_program()
    return run_bass_kernel_spmd(nc, in_maps, core_ids=list(range(NCORES)), **kwargs)


def make_in_maps(output, target):
    import ml_dtypes

    bf16 = ml_dtypes.bfloat16
    output = np.asarray(output, dtype=np.float32)
    target = np.asarray(target, dtype=np.float32)
    wband, wident = _make_consts()
    in_maps = []
    tpad = np.zeros((B, IMH, PADW), dtype=bf16)
    tpad[:, :, 32:1056] = target[:, 0].astype(bf16)
    o16 = output[:, 0].astype(bf16)
    wband16 = wband.astype(bf16)
    wident16 = wident.astype(bf16)
    for c in range(NCORES):
        in_maps.append(
            {
                "target": tpad[c * IMGS : (c + 1) * IMGS],
                "outp": np.ascontiguousarray(o16[c * IMGS : (c + 1) * IMGS]),
                "wband": wband16,
                "wident": wident16,
            }
        )
    return in_maps


def finish_on_host(results):
    losses = []
    for c in range(NCORES):
        st = np.asarray(results[c]["stats"], dtype=np.float64)  # [IMGS, 3, 512]
        s = st.sum(axis=2)                                      # [IMGS, 3]
        for g in range(IMGS):
            su, sv, sp2 = s[g]
            inter = sp2
            mask = su + sv
            losses.append(1.0 - (2.0 * inter + SMOOTH) / (mask + SMOOTH))
    return np.float32(np.mean(losses))


def kernel(output, target):
    in_maps = make_in_maps(output, target)
    res = run_on_device(in_maps)
    return finish_on_host(res.results)


# revision 21
# speedup vs baseline: 1.2268x; 1.0691x over previous
"""DiceLoss kernel for Trainium2 (8 NeuronCores, batch-sharded).

Math per image (H=W=1024):
    s  = sigmoid(output)
    P  = avgpool31x31(target)            (zero-padded box sum / 961)
    w  = 1 + 5*|P - target|
    inter = sum(s*t*w);  mask = sum((t+s)*w)
    loss  = 1 - (2*inter + 1e-3) / (mask + 1e-3);  final = mean over batch

Device strategy (per core: 4 images, 8 row-tiles of [128, 1024] each,
processed in pairs of row-tiles = [128, 2048] working set). All inputs
arrive as bf16 (host casts), halving HBM traffic:
  - horizontal 31-tap box: ONE tensor_tensor_scan per row-tile
        state = (t[x+15] + state) - t[x-16]   (fp32 state) over padded rows
  - vertical 31-tap box and the "- 961*t" subtraction: PE band matmuls
        psum = Bdiag@H_i + Babove@H_{i-1} + Bbelow@H_{i+1}
               - 960*I@t_i - 1*I@t_i        (both exact in bf16)
  - w1 = |5/961 * psum| on ACT (scale fused into Abs), bf16 out
  - fused DVE products with built-in row reductions (accum_out):
        u  = (w1+1)*t   with accum -> sum(t*w)    [scalar_tensor_tensor]
        v  = (w1+1)*s   with accum -> sum(s*w)    [scalar_tensor_tensor]
        p2 = s*u        with accum -> sum(s*t*w)  [tensor_tensor_reduce]
  - per-pair per-partition sums land in a [128, 48] f32 stash; host does
    the final cross-partition/cross-pair sums and the loss arithmetic.
No GPSIMD ops (they run ~25x below spec and lock the SBUF port shared
with DVE), no on-device casts, no PE reduction matmuls.
"""

import numpy as np

B, IMH, IMW = 32, 1024, 1024
NCORES = 8
IMGS = B // NCORES  # 4 images per core
NT = IMH // 128     # 8 row-tiles per image
NPAIR = NT // 2     # 4 pairs per image
PADW = 1072         # 32 zeros | 1024 data | 16 zeros (host-padded; 4B-aligned data)
SMOOTH = 1e-3

_CACHE = {}


def _make_consts():
    k = np.arange(128)[:, None]  # lhsT row (contraction index within rhs block)
    m = np.arange(128)[None, :]  # lhsT col (output partition within out block)
    diag = (np.abs(m - k) <= 15).astype(np.float32)
    above = (np.abs(128 + m - k) <= 15).astype(np.float32)   # rhs = H_{i-1}
    below = (np.abs(-128 + m - k) <= 15).astype(np.float32)  # rhs = H_{i+1}
    wband = np.stack([diag, above, below])                   # [3,128,128]
    wident = (-961.0 * np.eye(128)).astype(np.float32)       # exact in fp16
    return wband, wident


def _build_program():
    from contextlib import ExitStack

    import concourse.bacc as bacc
    import concourse.tile as tile
    from concourse import mybir

    f32 = mybir.dt.float32
    bf16 = mybir.dt.bfloat16
    f16 = mybir.dt.float16
    AL = mybir.AluOpType
    AF = mybir.ActivationFunctionType

    nc = bacc.Bacc("TRN2", target_bir_lowering=False, debug=False)
    t_dram = nc.dram_tensor("target", [IMGS, IMH, PADW], bf16, kind="ExternalInput")
    o_dram = nc.dram_tensor("outp", [IMGS, IMH, IMW], bf16, kind="ExternalInput")
    wb_dram = nc.dram_tensor("wband", [3, 128, 128], bf16, kind="ExternalInput")
    wi_dram = nc.dram_tensor("wident", [128, 128], f16, kind="ExternalInput")
    st_dram = nc.dram_tensor("stats", [IMGS, 3, 512], f32, kind="ExternalOutput")

    with tile.TileContext(nc) as tc, ExitStack() as ctx:
        consts = ctx.enter_context(tc.tile_pool(name="consts", bufs=1))
        tpool = ctx.enter_context(tc.tile_pool(name="tbuf", bufs=4))
        opool = ctx.enter_context(tc.tile_pool(name="obuf", bufs=4))
        hpool = ctx.enter_context(tc.tile_pool(name="hbuf", bufs=6))
        spool = ctx.enter_context(tc.tile_pool(name="sbuf16", bufs=4))
        w1pool = ctx.enter_context(tc.tile_pool(name="w1buf", bufs=2))
        wtpool = ctx.enter_context(tc.tile_pool(name="wtbuf", bufs=2))
        upool = ctx.enter_context(tc.tile_pool(name="ubuf", bufs=2))
        junkpool = ctx.enter_context(tc.tile_pool(name="junk", bufs=2))
        psd = ctx.enter_context(tc.tile_pool(name="psd", bufs=2, space="PSUM"))
        psr = ctx.enter_context(tc.tile_pool(name="psr", bufs=1, space="PSUM"))
        stpool = ctx.enter_context(tc.tile_pool(name="stout", bufs=2))

        wband = consts.tile([128, 3, 128], bf16, tag="wband")
        nc.sync.dma_start(wband[:], wb_dram.rearrange("a b c -> b a c"))
        wident = consts.tile([128, 128], f16, tag="wident")
        nc.sync.dma_start(wident[:], wi_dram[:])
        ones16 = consts.tile([128, 1], bf16, tag="ones16")
        nc.vector.memset(ones16[:], 1.0)

        # ---- pipeline over (image, pair) with 1-pair lag ----
        seq = [(g, p) for g in range(IMGS) for p in range(NPAIR)]
        tbs, hbs, sbs = {}, {}, {}
        red = {}  # per-image [1,3,512] psum accumulators

        def emit_load(idx, g, p):
            tb = tpool.tile([128, 2, PADW], bf16, tag="tb")
            tsrc = t_dram[g, 256 * p : 256 * (p + 1), :].rearrange(
                "(two r) w -> r two w", two=2
            )
            nc.sync.dma_start(tb[:], tsrc)
            ob = opool.tile([128, 2, 1024], bf16, tag="ob")
            osrc = o_dram[g, 256 * p : 256 * (p + 1), :].rearrange(
                "(two r) w -> r two w", two=2
            )
            nc.sync.dma_start(ob[:], osrc)

            hb = hpool.tile([128, 2, 1040], bf16, tag="hb")
            for k in range(2):
                # out[c] = CUM(c) - CUM(c-31); cols 15+x hold H[x]
                nc.vector.tensor_tensor_scan(
                    out=hb[:, k, 0:1039],
                    data0=tb[:, k, 32:1071],
                    data1=tb[:, k, 1:1040],
                    initial=0.0,
                    op0=AL.add,
                    op1=AL.subtract,
                )
            sb = spool.tile([128, 2, 1024], bf16, tag="sb")
            nc.scalar.activation(sb[:], ob[:], AF.Sigmoid)
            tbs[idx], hbs[idx], sbs[idx] = tb, hb, sb

        def h_view(base, j, h):
            # H row-tile j (pair handle at seq index base + j//2), half h
            return hbs[base + j // 2][:, j % 2, 15 + 512 * h : 15 + 512 * (h + 1)]

        def emit_process(idx, g, p):
            base = idx - p  # seq index of this image's pair 0
            tb = tbs[idx]
            sb = sbs[idx]
            w1 = w1pool.tile([128, 2, 1024], bf16, tag="w1")
            for k in range(2):
                j = 2 * p + k
                dps = psd.tile([128, 2, 512], f32, tag="dps")
                for h in range(2):
                    nc.tensor.matmul(
                        dps[:, h, :], wband[:, 0, :], h_view(base, j, h),
                        start=True, stop=False,
                    )
                    if j > 0:
                        nc.tensor.matmul(
                            dps[:, h, :], wband[:, 1, :], h_view(base, j - 1, h),
                            start=False, stop=False,
                        )
                    if j < NT - 1:
                        nc.tensor.matmul(
                            dps[:, h, :], wband[:, 2, :], h_view(base, j + 1, h),
                            start=False, stop=False,
                        )
                    tview = tb[:, k, 32 + 512 * h : 32 + 512 * (h + 1)]
                    nc.tensor.matmul(
                        dps[:, h, :], wident[:], tview, start=False, stop=True
                    )
                nc.scalar.activation(
                    out=w1[:, k, :],
                    in_=dps[:].rearrange("p a b -> p (a b)"),
                    func=AF.Abs,
                    scale=5.0 / 961.0,
                )

            wt = wtpool.tile([128, 2, 1024], bf16, tag="wt")
            nc.scalar.add(wt[:], w1[:], 1.0)
            tf = tb[:, :, 32:1056]
            u = upool.tile([128, 2, 1024], bf16, tag="u")
            nc.vector.tensor_mul(u[:], tf, wt[:])
            v = junkpool.tile([128, 2, 1024], bf16, tag="v")
            nc.vector.tensor_mul(v[:], sb[:], wt[:])
            p2 = junkpool.tile([128, 2, 1024], bf16, tag="p2")
            nc.vector.tensor_mul(p2[:], sb[:], u[:])

            if p == 0:
                red[g] = psr.tile([1, 3, 512], f32, name=f"red{g}", tag="red")
            for q, src in enumerate((u, v, p2)):
                sf = src[:].rearrange("p a b -> p (a b)")
                for c in range(4):
                    nc.tensor.matmul(
                        red[g][:, q, :],
                        ones16[:],
                        sf[:, 512 * c : 512 * (c + 1)],
                        start=(p == 0 and c == 0),
                        stop=(p == NPAIR - 1 and c == 3),
                        skip_group_check=True,
                    )
            if p == NPAIR - 1:
                st = stpool.tile([1, 3, 512], f32, tag="st")
                nc.scalar.copy(st[:], red[g][:])
                nc.sync.dma_start(st_dram[g : g + 1], st[:])

        for idx in range(len(seq) + 1):
            if idx < len(seq):
                emit_load(idx, *seq[idx])
            if idx >= 1:
                emit_process(idx - 1, *seq[idx - 1])

    nc.compile()
    return nc


def _get_program():
    key = "nc"
    if key not in _CACHE:
        _CACHE[key] = _build_program()
    return _CACHE[key]


def run_on_device(in_maps, **kwargs):
    from concourse.bass_utils import run_bass_kernel_spmd

    nc = _get_program()
    return run_bass_kernel_spmd(nc, in_maps, core_ids=list(range(NCORES)), **kwargs)


def make_in_maps(output, target):
    import ml_dtypes

    bf16 = ml_dtypes.bfloat16
    output = np.asarray(output, dtype=np.float32)
    target = np.asarray(target, dtype=np.float32)
    wband, wident = _make_consts()
    in_maps = []
    tpad = np.zeros((B, IMH, PADW), dtype=bf16)
    tpad[:, :, 32:1056] = target[:, 0].astype(bf16)
    o16 = output[:, 0].astype(bf16)
    wband16 = wband.astype(bf16)
    wident16 = wident.astype(np.float16)
    for c in range(NCORES):
        in_maps.append(
            {
                "target": tpad[c * IMGS : (c + 1) * IMGS],
                "outp": np.ascontiguousarray(o16[c * IMGS : (c + 1) * IMGS]),
                "wband": wband16,
                "wident": wident16,
            }
        )
    return in_maps


def finish_on_host(results):
    losses = []
    for c in range(NCORES):
        st = np.asarray(results[c]["stats"], dtype=np.float64)  # [IMGS, 3, 512]
        s = st.sum(axis=2)                                      # [IMGS, 3]
        for g in range(IMGS):
            su, sv, sp2 = s[g]
            inter = sp2
            mask = su + sv
            losses.append(1.0 - (2.0 * inter + SMOOTH) / (mask + SMOOTH))
    return np.float32(np.mean(losses))


def kernel(output, target):
    in_maps = make_in_maps(output, target)
    res = run_on_device(in_maps)
    return finish_on_host(res.results)
